# revision 1
# baseline (speedup 1.0000x reference)
"""CrossAttentionBlock3D on 8 Trainium2 NeuronCores.

Sharding: head-parallel (tensor parallel). Core i computes head i end to end:
  - GroupNorm is algebraically folded into the q/kv GEMM weights (per-channel
    scale a_c = w_c/sqrt(var_g+eps) and shift b_c = b_c - a_c*mu_g; only the
    group statistics are computed on device, in one streaming pass).
  - q = (q_w . diag(a)) @ x, kv = (kv_w . diag(a_ctx)) @ ctx  (fp32r matmuls)
  - logits^T tiles [ks,qs] on PE, exp on ACT (no max subtraction: logits have
    std ~0.2 for this problem's data, |logit| < ~2), PV matmul consumes exp
    tiles directly as stationary-v @ streaming-exp with an appended ones
    column producing the softmax denominator for free.
  - proj partial (contraction over this head's 64 channels) + per-core-zeroed
    proj bias + residual-scale vector (core 0 only) -> partial output.
Host sums the 8 partial outputs (the all-reduce of the tensor-parallel proj).
"""

import os
import sys

import numpy as np

for _p in ("/opt/trn_rl_repo",):
    if _p not in sys.path and os.path.isdir(_p):
        sys.path.insert(0, _p)

from contextlib import ExitStack

import concourse.bacc as bacc
import concourse.bass as bass
import concourse.tile as tile
from concourse import mybir
from concourse import masks
from concourse.bass_utils import run_bass_kernel_spmd

F32 = mybir.dt.float32
F32R = mybir.dt.float32r
BF16 = mybir.dt.bfloat16
AF = mybir.ActivationFunctionType
ALU = mybir.AluOpType
AX = mybir.AxisListType

C = 512          # channels
S = 4096         # spatial tokens (16*16*16)
HD = 64          # head dim
N_CORES = 8
EPS = 1e-5
NBLK = 8         # qs blocks
BLK = 512        # qs block width
KT = 32          # ks tiles of 128
GN = 262144.0    # elements per group (64 ch * 4096)


def _build_kernel(ctx: ExitStack, tc, t, out_ap):
    nc = tc.nc

    persist = ctx.enter_context(tc.tile_pool(name="persist", bufs=1))
    stat = ctx.enter_context(tc.tile_pool(name="stat", bufs=1))

    # ---- persistent SBUF tensors -------------------------------------------
    X = [persist.tile([128, S], F32, tag=f"x{k}", name=f"x{k}") for k in range(4)]
    qs_sb = persist.tile([64, S], BF16, tag="qs_sb", name="qs_sb")
    ks_sb = persist.tile([64, S], BF16, tag="ks_sb", name="ks_sb")
    v_aug = persist.tile([128, KT, HD + 1], BF16, tag="v_aug", name="v_aug")
    qwt = [persist.tile([128, 64], BF16, tag=f"qwt{k}", name=f"qwt{k}") for k in range(4)]
    kvwt = [persist.tile([128, 128], BF16, tag=f"kvwt{k}", name=f"kvwt{k}") for k in range(4)]
    pwt = persist.tile([64, C], BF16, tag="pwt", name="pwt")
    pb_sb = persist.tile([128, 4], F32, tag="pb_sb", name="pb_sb")
    rvec = persist.tile([128, 1], F32, tag="rvec", name="rvec")
    nw_sb = persist.tile([128, 8], F32, tag="nw_sb", name="nw_sb")
    nb_sb = persist.tile([128, 8], F32, tag="nb_sb", name="nb_sb")
    qb_sb = persist.tile([64, 1], F32, tag="qb_sb", name="qb_sb")
    kvb_sb = persist.tile([128, 1], F32, tag="kvb_sb", name="kvb_sb")
    qbe = persist.tile([64, 1], F32, tag="qbe", name="qbe")
    kvbe = persist.tile([128, 1], F32, tag="kvbe", name="kvbe")
    ident = persist.tile([64, 64], BF16, tag="ident", name="ident")

    masks.make_identity(nc, ident[:])
    nc.vector.memset(v_aug[:, :, HD : HD + 1], 1.0)

    # ---- load weights / small tensors --------------------------------------
    for k in range(4):
        nc.gpsimd.dma_start(qwt[k][:], t["qwt"][k * 128 : (k + 1) * 128, :])
        nc.gpsimd.dma_start(kvwt[k][:], t["kvwt"][k * 128 : (k + 1) * 128, :])
    nc.gpsimd.dma_start(pwt[:], t["pwt"][:])
    nc.sync.dma_start(pb_sb[:], t["pb"][:])
    nc.sync.dma_start(rvec[:], t["rvec"][:])
    nc.sync.dma_start(qb_sb[:], t["qb"][:])
    nc.sync.dma_start(kvb_sb[:], t["kvb"][:])
    nc.sync.dma_start(nw_sb[:, 0:4], t["nwx"][:])
    nc.sync.dma_start(nw_sb[:, 4:8], t["nwc"][:])
    nc.sync.dma_start(nb_sb[:, 0:4], t["nbx"][:])
    nc.sync.dma_start(nb_sb[:, 4:8], t["nbc"][:])

    # ---- phase 0/1: stream x and ctx in, per-chunk sums and sum-of-squares --
    ctx_es = ExitStack()
    ctx_pool = ctx_es.enter_context(tc.tile_pool(name="ctx_pool", bufs=1))
    CX = [ctx_pool.tile([128, S], BF16, tag=f"c{k}", name=f"c{k}") for k in range(4)]
    XB = [ctx_pool.tile([128, S], BF16, tag=f"xb{k}", name=f"xb{k}") for k in range(4)]
    for k in range(4):
        nc.sync.dma_start(X[k][:], t["x"][k * 128 : (k + 1) * 128, :])
        nc.gpsimd.dma_start(XB[k][:], t["x"][k * 128 : (k + 1) * 128, :])
    for k in range(4):
        nc.gpsimd.dma_start(CX[k][:], t["ctx"][k * 128 : (k + 1) * 128, :])

    stats16 = stat.tile([128, 16], F32, tag="stats16", name="stats16")
    with tc.tile_pool(name="scratch", bufs=2) as scratch:
        for j, src in enumerate(XB + CX):
            nc.vector.reduce_sum(stats16[:, j : j + 1], src[:], axis=AX.X)
            sc = scratch.tile([128, S], F32, tag="scr", name=f"scr{j}")
            nc.scalar.activation(
                sc[:], src[:], AF.Square, accum_out=stats16[:, 8 + j : 9 + j]
            )

    # ---- group-stat combine via tiny PE matmuls -----------------------------
    halfind = stat.tile([128, 2], F32, tag="halfind", name="halfind")
    nc.sync.dma_start(halfind[:], t["halfind"][:])
    bcast2 = stat.tile([2, 128], F32, tag="bcast2", name="bcast2")
    nc.sync.dma_start(bcast2[:], t["bcast2"][:])

    with tc.tile_pool(name="ps_tiny", bufs=1, space="PSUM") as ps_tiny:
        g1 = ps_tiny.tile([2, 16], F32, tag="g1", name="g1")
        nc.tensor.matmul(g1[:], lhsT=halfind[:], rhs=stats16[:], start=True, stop=True)
        g1s = stat.tile([2, 16], F32, tag="g1s", name="g1s")
        nc.vector.tensor_copy(g1s[:], g1[:])
        g2 = ps_tiny.tile([128, 16], F32, tag="g2", name="g2")
        nc.tensor.matmul(g2[:], lhsT=bcast2[:], rhs=g1s[:], start=True, stop=True)
        pcs = stat.tile([128, 16], F32, tag="pcs", name="pcs")
        nc.vector.tensor_copy(pcs[:], g2[:])

        mean = stat.tile([128, 8], F32, tag="mean", name="mean")
        nc.vector.tensor_scalar_mul(mean[:], pcs[:, 0:8], 1.0 / GN)
        var = stat.tile([128, 8], F32, tag="var", name="var")
        nc.vector.tensor_scalar_mul(var[:], pcs[:, 8:16], 1.0 / GN)
        m2 = stat.tile([128, 8], F32, tag="m2", name="m2")
        nc.vector.tensor_mul(m2[:], mean[:], mean[:])
        nc.vector.tensor_sub(var[:], var[:], m2[:])
        # rstd = exp(-0.5*ln(var+eps)) keeps everything in one ACT table set
        eps_t = stat.tile([128, 1], F32, tag="eps_t", name="eps_t")
        nc.vector.memset(eps_t[:], EPS)
        lnv = stat.tile([128, 8], F32, tag="lnv", name="lnv")
        nc.scalar.activation(lnv[:], var[:], AF.Ln, bias=eps_t[:])
        rstd = stat.tile([128, 8], F32, tag="rstd", name="rstd")
        nc.scalar.activation(rstd[:], lnv[:], AF.Exp, scale=-0.5)
        a_sc = stat.tile([128, 8], F32, tag="a_sc", name="a_sc")
        nc.vector.tensor_mul(a_sc[:], nw_sb[:], rstd[:])
        bsh = stat.tile([128, 8], F32, tag="bsh", name="bsh")
        nc.vector.tensor_mul(bsh[:], a_sc[:], mean[:])
        nc.vector.tensor_sub(bsh[:], nb_sb[:], bsh[:])

        bsh_bf = stat.tile([128, 8], BF16, tag="bsh_bf", name="bsh_bf")
        nc.vector.tensor_copy(bsh_bf[:], bsh[:])
        # effective q/kv biases: b + W @ b_shift (with unfolded W), then fold W
        qeb = ps_tiny.tile([64, 1], F32, tag="qeb", name="qeb")
        kveb = ps_tiny.tile([128, 1], F32, tag="kveb", name="kveb")
        for k in range(4):
            nc.tensor.matmul(
                qeb[:], lhsT=qwt[k][:], rhs=bsh_bf[:, k : k + 1],
                start=(k == 0), stop=(k == 3),
            )
            nc.tensor.matmul(
                kveb[:], lhsT=kvwt[k][:], rhs=bsh_bf[:, 4 + k : 5 + k],
                start=(k == 0), stop=(k == 3),
            )
        nc.vector.tensor_add(qbe[:], qb_sb[:], qeb[:])
        nc.vector.tensor_add(kvbe[:], kvb_sb[:], kveb[:])
        for k in range(4):
            nc.vector.tensor_scalar_mul(qwt[k][:], qwt[k][:], a_sc[:, k : k + 1])
            nc.vector.tensor_scalar_mul(kvwt[k][:], kvwt[k][:], a_sc[:, 4 + k : 5 + k])

    # ---- phase 2: q / kv GEMMs (kv columns ordered v|k) ---------------------
    vcs_es = ExitStack()
    v_cs_pool = vcs_es.enter_context(tc.tile_pool(name="v_cs_pool", bufs=1))
    v_cs = v_cs_pool.tile([64, S], BF16, tag="v_cs", name="v_cs")
    with tc.tile_pool(name="ps_gemm", bufs=2, space="PSUM") as ps_gemm:
        for b in range(NBLK):
            qs = slice(b * BLK, (b + 1) * BLK)
            qp = ps_gemm.tile([64, BLK], F32, tag="qp", name=f"qp{b}")
            for k in range(4):
                nc.tensor.matmul(
                    qp[:], lhsT=qwt[k][:], rhs=XB[k][:, qs],
                    start=(k == 0), stop=(k == 3),
                )
            nc.vector.tensor_scalar(
                qs_sb[:, qs], qp[:], scalar1=qbe[:], scalar2=None, op0=ALU.add
            )
            kvp = ps_gemm.tile([128, BLK], F32, tag="kvp", name=f"kvp{b}")
            for k in range(4):
                nc.tensor.matmul(
                    kvp[:], lhsT=kvwt[k][:], rhs=CX[k][:, qs],
                    start=(k == 0), stop=(k == 3),
                )
            nc.vector.tensor_scalar(
                v_cs[:, qs], kvp[0:64, :], scalar1=kvbe[0:64], scalar2=None, op0=ALU.add
            )
            nc.vector.tensor_scalar(
                ks_sb[:, qs], kvp[64:128, :], scalar1=kvbe[64:128],
                scalar2=None, op0=ALU.add,
            )

    # ---- phase 2.5: transpose v to [s, c] with PE, append ones --------------
    with tc.tile_pool(name="ps_tr", bufs=2, space="PSUM") as ps_tr:
        for kt in range(KT):
            ptr = ps_tr.tile([128, 64], BF16, tag="tr", name=f"tr{kt}")
            nc.tensor.transpose(ptr[:], v_cs[:, kt * 128 : (kt + 1) * 128], ident[:])
            nc.vector.tensor_copy(v_aug[:, kt, 0:HD], ptr[:])
    # ctx and v_cs are dead from here on; free their SBUF for the exp tiles
    vcs_es.close()
    ctx_es.close()

    # ---- phase 3: attention + proj, per qs block ----------------------------
    exp_pool = ctx.enter_context(tc.tile_pool(name="exp_pool", bufs=3))
    o2_pool = ctx.enter_context(tc.tile_pool(name="o2_pool", bufs=2))
    stage_pool = ctx.enter_context(tc.tile_pool(name="stage_pool", bufs=4))
    ps_lg = ctx.enter_context(tc.tile_pool(name="ps_lg", bufs=2, space="PSUM"))
    ps_pv = ctx.enter_context(tc.tile_pool(name="ps_pv", bufs=1, space="PSUM"))
    ps_pj = ctx.enter_context(tc.tile_pool(name="ps_pj", bufs=1, space="PSUM"))

    # 32 ks tiles -> ACT chunks of 3 tiles (1536 wide) + one final 2-tile chunk
    chunk_sizes = [3] * 10 + [2]
    for b in range(NBLK):
        qs = slice(b * BLK, (b + 1) * BLK)
        pv = ps_pv.tile([HD + 1, BLK], F32, tag="pv", name=f"pv{b}")
        kt0 = 0
        for ci, csz in enumerate(chunk_sizes):
            w = csz * BLK
            lg = ps_lg.tile([128, 1536], F32, tag="lg", name=f"lg{b}_{ci}")
            et = exp_pool.tile([128, 1536], BF16, tag="et", name=f"et{b}_{ci}")
            for i in range(csz):
                kt = kt0 + i
                nc.tensor.matmul(
                    lg[:, i * BLK : (i + 1) * BLK],
                    lhsT=ks_sb[:, kt * 128 : (kt + 1) * 128],
                    rhs=qs_sb[:, qs],
                    start=True, stop=True,
                )
            nc.scalar.activation(et[:, 0:w], lg[:, 0:w], AF.Exp, scale=0.125)
            for i in range(csz):
                kt = kt0 + i
                nc.tensor.matmul(
                    pv[:],
                    lhsT=v_aug[:, kt, :],
                    rhs=et[:, i * BLK : (i + 1) * BLK],
                    start=(kt == 0), stop=(kt == KT - 1),
                    skip_group_check=True,
                )
            kt0 += csz

        o2 = o2_pool.tile([HD + 1, BLK], F32, tag="o2", name=f"o2{b}")
        nc.vector.tensor_copy(o2[:], pv[:])
        rd = o2_pool.tile([1, BLK], F32, tag="rd", name=f"rd{b}")
        nc.vector.reciprocal(rd[:], o2[HD : HD + 1, :])
        bc = o2_pool.tile([64, BLK], F32, tag="bc", name=f"bc{b}")
        nc.gpsimd.partition_broadcast(bc[:], rd[:])
        o2n = o2_pool.tile([64, BLK], BF16, tag="o2n", name=f"o2n{b}")
        nc.vector.tensor_mul(o2n[:], o2[0:HD, :], bc[:])

        for oc in range(4):
            pj = ps_pj.tile([128, BLK], F32, tag="pj", name=f"pj{b}_{oc}")
            nc.tensor.matmul(
                pj[:],
                lhsT=pwt[:, oc * 128 : (oc + 1) * 128],
                rhs=o2n[:],
                start=True, stop=True,
            )
            st = stage_pool.tile([128, BLK], F32, tag="st", name=f"st{b}_{oc}")
            # + proj bias (zeroed on cores != 0)
            nc.vector.tensor_scalar(
                st[:], pj[:], scalar1=pb_sb[:, oc : oc + 1], scalar2=None, op0=ALU.add
            )
            # + residual r*x (r = 1 on core 0, 0 elsewhere)
            nc.vector.scalar_tensor_tensor(
                st[:], in0=X[oc][:, qs], scalar=rvec[:], in1=st[:],
                op0=ALU.mult, op1=ALU.add,
            )
            nc.sync.dma_start(out_ap[oc * 128 : (oc + 1) * 128, qs], st[:])


_CACHED = {}


def _build_program():
    if "nc" in _CACHED:
        return _CACHED["nc"]
    nc = bacc.Bacc("TRN2", target_bir_lowering=False, debug=False,
                   num_devices=N_CORES)
    t = {}

    def inp(name, shape):
        t[name] = nc.dram_tensor(name, shape, F32, kind="ExternalInput").ap()

    inp("x", [C, S])
    inp("ctx", [C, S])
    inp("qwt", [C, HD])
    inp("qb", [HD, 1])
    inp("kvwt", [C, 2 * HD])
    inp("kvb", [2 * HD, 1])
    inp("pwt", [HD, C])
    inp("pb", [128, 4])
    inp("rvec", [128, 1])
    inp("nwx", [128, 4])
    inp("nbx", [128, 4])
    inp("nwc", [128, 4])
    inp("nbc", [128, 4])
    inp("halfind", [128, 2])
    inp("bcast2", [2, 128])
    out_ap = nc.dram_tensor("out", [C, S], F32, kind="ExternalOutput").ap()

    with tile.TileContext(nc) as tc:
        with ExitStack() as es:
            _build_kernel(es, tc, t, out_ap)
    nc.compile()
    _CACHED["nc"] = nc
    return nc


def make_in_maps(**inputs):
    """Build the 8 per-core input dicts from the full problem inputs."""
    f = lambda v: np.ascontiguousarray(np.asarray(v), dtype=np.float32)
    x = f(inputs["x"]).reshape(C, S)
    cx = f(inputs["context"]).reshape(C, S)
    q_w, q_b = f(inputs["q_w"]), f(inputs["q_b"])
    kv_w, kv_b = f(inputs["kv_w"]), f(inputs["kv_b"])
    p_w, p_b = f(inputs["proj_w"]), f(inputs["proj_b"])
    k_w, v_w = kv_w[:C], kv_w[C:]
    k_b, v_b = kv_b[:C], kv_b[C:]
    vec4 = lambda v: np.ascontiguousarray(v.reshape(4, 128).T)
    nwx, nbx = vec4(f(inputs["norm_w"])), vec4(f(inputs["norm_b"]))
    nwc, nbc = vec4(f(inputs["normc_w"])), vec4(f(inputs["normc_b"]))
    pb4 = vec4(p_b)
    halfind = np.zeros((128, 2), np.float32)
    halfind[0:64, 0] = 1.0
    halfind[64:128, 1] = 1.0
    bcast2 = np.ascontiguousarray(halfind.T)

    in_maps = []
    for i in range(N_CORES):
        hs = slice(i * HD, (i + 1) * HD)
        core0 = i == 0
        in_maps.append({
            "x": x,
            "ctx": cx,
            "qwt": np.ascontiguousarray(q_w[hs].T),
            "qb": np.ascontiguousarray(q_b[hs].reshape(HD, 1)),
            "kvwt": np.ascontiguousarray(
                np.concatenate([v_w[hs], k_w[hs]], axis=0).T),
            "kvb": np.ascontiguousarray(
                np.concatenate([v_b[hs], k_b[hs]]).reshape(2 * HD, 1)),
            "pwt": np.ascontiguousarray(p_w[:, hs].T),
            "pb": pb4 if core0 else np.zeros((128, 4), np.float32),
            "rvec": (np.ones if core0 else np.zeros)((128, 1), np.float32),
            "nwx": nwx, "nbx": nbx, "nwc": nwc, "nbc": nbc,
            "halfind": halfind, "bcast2": bcast2,
        })
    return in_maps


def kernel(**inputs):
    nc = _build_program()
    in_maps = make_in_maps(**inputs)
    res = run_bass_kernel_spmd(nc, in_maps, list(range(N_CORES)))
    out = np.zeros((C, S), np.float64)
    for r in res.results:
        out += r["out"].astype(np.float64)
    return out.astype(np.float32).reshape(1, C, 16, 16, 16)


if __name__ == "__main__":
    nc = _build_program()
    print("program built ok")

